# revision 37
# baseline (speedup 1.0000x reference)
"""Trainium2 Bass kernel for nn_Circuit_26654567039463.

Integrates dA/dt = i(omega + nu|A|^2)A + A @ T2t for a batch of 2048
trajectories (data-parallel over 8 NeuronCores, 256 per core), matching
the reference's fixed-step dopri5 (99 intervals x 5 substeps).

Scheme (host-validated, rel err ~3.6e-3 vs the jax reference):
the dopri5 map is linear (M0 per substep) plus a small nonlinear phase
theta = h*nu*|A|^2 per substep.  Each macro step advances TWO intervals:
    y_{i+1} = M10 y_i + C75 q0 + C25 q1        (chain, interval 2i+2)
    out     = M5  y_i + C25 q0                 (branch, interval 2i+1)
with one gate node per interval (q_j = theta ⊙ s_j at substep midpoints
2.5/7.5 of the macro; quadrature over the 5 substeps of an interval is
insensitive to node count).  The node states s_j are linearly
extrapolated from two stale predictions (3*P@y_{i-2} - 2*P'@y_{i-3}),
and theta comes from a single shared-position prediction (lag 8
intervals) — staleness of theta is cheap because |A|^2 is insensitive
to the missed nonlinear phase.  All gate math runs one macro ahead of
the state chain, so the only per-macro critical path is
matmul -> PSUM->SBUF copy.
"""
import sys
for _p in ("/opt/trn_rl_repo",):
    if _p not in sys.path:
        sys.path.insert(0, _p)

import numpy as np

import concourse.mybir as mybir
import concourse.tile as tile
from concourse import bacc

F32 = mybir.dt.float32
F32R = mybir.dt.float32r

MODES, INPUT_MODES, EVAL_PTS, T_END, SUBSTEPS = 64, 48, 100, 0.5, 5
N_INTERVALS_FULL = EVAL_PTS - 1
DT = T_END / (EVAL_PTS - 1)
H = DT / SUBSTEPS
B_CORE = 256  # batch per core
N_MACRO = 49  # macros 0..48 cover intervals 1..98; interval 99 is epilogue

ATAB = {
    (2, 1): 0.2,
    (3, 1): 0.075, (3, 2): 0.225,
    (4, 1): 44 / 45, (4, 2): -56 / 15, (4, 3): 32 / 9,
    (5, 1): 19372 / 6561, (5, 2): -25360 / 2187, (5, 3): 64448 / 6561, (5, 4): -212 / 729,
    (6, 1): 9017 / 3168, (6, 2): -355 / 33, (6, 3): 46732 / 5247, (6, 4): 49 / 176,
    (6, 5): -5103 / 18656,
    (7, 1): 35 / 384, (7, 2): 0.0, (7, 3): 500 / 1113, (7, 4): 125 / 192,
    (7, 5): -2187 / 6784, (7, 6): 11 / 84,
}


# ---------------------------------------------------------------- host math
def make_T2(params, kappa, dtype=np.complex128):
    n = MODES
    M = np.concatenate([params, np.zeros((1,), params.dtype)]).reshape(n, n)
    Hh = 0.5 * (M + M.T)
    iH = (1j * Hh).astype(dtype)
    eye = np.eye(n, dtype=dtype)
    U = np.linalg.solve(eye + iH, eye - iH)
    UtU = U.T @ U
    mix = UtU @ np.linalg.inv(eye - UtU + np.array(1e-8, dtype) * eye)
    return -kappa[None, :].astype(dtype) * (0.5 * eye + mix)


def real_rep(M):
    """Real [128,128] rep of complex a -> M a (state layout [Re; Im])."""
    Mr, Mi = M.real, M.imag
    return np.block([[Mr.T, -Mi.T], [Mi.T, Mr.T]])


def dopri_linear_map(Lx):
    """Zeroth-order dopri5 step map for y' -> M y given L = h*W."""
    n2 = Lx.shape[0]
    I = np.eye(n2)
    K0 = {}
    for i in range(1, 7):
        Pi = I.copy()
        for l in range(1, i):
            Pi = Pi + ATAB[(i, l)] * K0[l]
        K0[i] = Lx @ Pi
    M = I.copy()
    for i in range(1, 7):
        M = M + ATAB[(7, i)] * K0[i]
    return M


def build_weights(params, kappa, omega, nonlinearity=None):
    """Returns (wmats [NW,128,128] f32 as lhsT, index map)."""
    if nonlinearity is None:
        nonlinearity = np.full((MODES,), 0.2, np.float32)
    scv = np.sqrt(H * nonlinearity.astype(np.float64))
    scv = np.concatenate([scv, scv])  # [128] per-partition sqrt(H*nu)
    T2 = make_T2(params.astype(np.float64), kappa.astype(np.float64))
    Wt = H * (T2.T + 1j * np.diag(omega.astype(np.float64)))
    L = real_rep(Wt)
    M0 = dopri_linear_map(L)
    M0h = dopri_linear_map(L * 0.5)
    J = np.block([[np.zeros((64, 64)), -np.eye(64)],
                  [np.eye(64), np.zeros((64, 64))]])

    def Mp(k):
        return np.linalg.matrix_power(M0, k)

    def Mh(k):  # M0^{k+0.5}
        return M0h @ Mp(k)

    M5 = Mp(5)
    mats = []
    idx = {}

    def add(name, X):
        idx[name] = len(mats)
        mats.append(np.ascontiguousarray(X.T))

    # head chunk (first N_HEAD mats): everything the prologue touches, so
    # a small fast DMA unblocks the PE immediately.  Theta-prediction mats
    # carry diag(sqrt(H*nu)) baked in, so sq needs no scale vector.
    i64 = np.eye(64)
    S = np.diag(scv)
    add("PR0", Mh(2))             # psE(0)
    add("PR1", Mh(7))
    add("PA0u", Mh(12))           # psE(1)
    add("PA1u", Mh(17))
    add("THP0", S @ M5)           # theta(0)
    add("THP1", S @ Mp(15))       # theta(1)
    add("THP2", S @ Mp(25))       # theta(2)
    add("THP3", S @ Mp(35))       # theta(3)
    add("fold", np.block([[i64, i64], [i64, i64]]))
    # rest chunk: steady-state weights (first used a few us in)
    add("M10", Mp(10))            # chain propagator
    add("M5", M5)                 # branch propagator
    add("C25", 5.0 * (Mh(2) @ J))
    add("C75", 5.0 * (Mh(7) @ J))
    add("PA3", 3.0 * Mh(22))      # psE from y_i
    add("PA3b", 3.0 * Mh(27))
    add("PB2", -2.0 * Mh(32))     # psE from y_{i-1}
    add("PB2b", -2.0 * Mh(37))
    add("TH", S @ Mp(45))         # theta prediction (macro i+4)
    add("PB0u", Mh(22))           # psE(2) prologue
    add("PB1u", Mh(27))
    # partition-major pack: one [128, NW*128] DMA loads every stationary
    wmats = np.stack(mats).astype(np.float32)
    wmats = np.ascontiguousarray(wmats.transpose(1, 0, 2).reshape(128, -1))
    return wmats, idx


def host_initial_state(A0_real, A0_imag, biases_real, biases_imag):
    """[128, B] mode-major initial padded state for a batch shard."""
    B = A0_real.shape[0]
    S = np.zeros((128, B), np.float32)
    S[:INPUT_MODES] = A0_real.T
    S[INPUT_MODES:MODES] = np.broadcast_to(biases_real[:, None], (MODES - INPUT_MODES, B))
    S[MODES:MODES + INPUT_MODES] = A0_imag.T
    S[MODES + INPUT_MODES:] = np.broadcast_to(biases_imag[:, None], (MODES - INPUT_MODES, B))
    return S


def host_scalevec(nonlinearity):
    s = np.sqrt(H * nonlinearity.astype(np.float64)).astype(np.float32)
    return np.concatenate([s, s]).reshape(128, 1)


# ---------------------------------------------------------------- kernel
def build_kernel(n_intervals, idx):
    assert n_intervals == N_INTERVALS_FULL
    NW = len(idx)
    nc = bacc.Bacc("TRN2")
    s0_d = nc.dram_tensor("s0", [128, B_CORE], F32R, kind="ExternalInput")
    w_d = nc.dram_tensor("wmats", [128, NW * 128], F32R, kind="ExternalInput")
    # partition-major layout: one combined DMA covers both macro outputs
    traj_d = nc.dram_tensor("traj", [128, n_intervals, B_CORE], F32R,
                            kind="ExternalOutput")

    with tile.TileContext(nc) as tc:
        import contextlib
        with contextlib.ExitStack() as ctx:
            singles = ctx.enter_context(tc.tile_pool(name="singles", bufs=1))
            # out tile: [0:256] branch output (interval 2i+1),
            #           [256:512] chain state y_{i+1} (interval 2i+2)
            out_p = ctx.enter_context(tc.tile_pool(name="out", bufs=6))
            thsb_p = ctx.enter_context(tc.tile_pool(name="thsb", bufs=4))
            sq_p = ctx.enter_context(tc.tile_pool(name="sq", bufs=4))
            q_p = ctx.enter_context(tc.tile_pool(name="q", bufs=4))
            psE_p = ctx.enter_context(tc.tile_pool(name="psE", bufs=2, space="PSUM"))
            # packed banks: [0:256] theta-prediction, [256:512] theta (fold)
            psG_p = ctx.enter_context(tc.tile_pool(name="psG", bufs=2, space="PSUM"))
            # chain and branch in SEPARATE banks: sharing one bank serializes
            # the branch matmuls behind the chain copy (bank-level hazard)
            psCh_p = ctx.enter_context(tc.tile_pool(name="psCh", bufs=2, space="PSUM"))
            psBr_p = ctx.enter_context(tc.tile_pool(name="psBr", bufs=2, space="PSUM"))

            # ---- one-time setup: the head chunk carries every warmup +
            # prologue stationary and goes FIRST so the PE unblocks early;
            # the big rest chunk rides a parallel Act-queue DMA
            N_HEAD = 9
            wt_head = singles.tile([128, N_HEAD * 128], F32R, tag="wt_head")
            nc.sync.dma_start(wt_head[:], w_d[:, 0:N_HEAD * 128])
            # s0 rides the DVE queue so its transfer isn't stuck behind the
            # big weight DMAs on the shared transfer stage; wt_rest goes LAST
            y0t = singles.tile([128, B_CORE], F32R, tag="y0")
            nc.scalar.dma_start(y0t[:], s0_d[:])
            wt_rest = singles.tile([128, (NW - N_HEAD) * 128], F32R,
                                   tag="wt_rest")
            nc.sync.dma_start(wt_rest[:], w_d[:, N_HEAD * 128:])
            wts = {}
            for name, i in idx.items():
                if i < N_HEAD:
                    wts[name] = wt_head[:, 128 * i:128 * (i + 1)]
                else:
                    wts[name] = wt_rest[:, 128 * (i - N_HEAD):
                                        128 * (i - N_HEAD + 1)]
            y = y0t

            # PE warm-up: ~10us of continuous PE activity flips the HAM
            # clock gate to full speed.  The junk matmuls read a memset
            # SBUF tile, so they start immediately without waiting for any
            # input DMA; they are interleaved with the prologue's real
            # matmuls so the warm-up window doubles as pipeline fill.
            jsrc_f = singles.tile([128, B_CORE], F32, tag="jsrc_f")
            nc.vector.memset(jsrc_f[:], 1.0)
            jsrc = singles.tile([128, B_CORE], F32R, tag="jsrc")
            nc.vector.tensor_copy(jsrc[:], jsrc_f[:])
            _junk_state = [0]

            def junk(n):
                for _ in range(n):
                    tag = "ch" if _junk_state[0] % 2 == 0 else "br"
                    pool = psCh_p if _junk_state[0] % 2 == 0 else psBr_p
                    jt = pool.tile([128, B_CORE], F32, tag=tag)
                    nc.tensor.matmul(jt[:], jsrc[:, 0:128], jsrc[:],
                                     start=True, stop=True)
                    _junk_state[0] += 1

            junk(10)

            def mk_sq(pred_wname, src, gt):
                """theta prediction into gt[0:256] -> sq (Act)."""
                nc.tensor.matmul(gt[:, 0:B_CORE], wts[pred_wname], src[:],
                                 start=True, stop=True)
                sq = sq_p.tile([128, B_CORE], F32R, tag="sq")
                nc.scalar.activation(sq[:], gt[:, 0:B_CORE],
                                     mybir.ActivationFunctionType.Square)
                return sq

            def mk_fold(sq, gt):
                nc.tensor.matmul(gt[:, B_CORE:], wts["fold"], sq[:],
                                 start=True, stop=True)
                return gt

            def mk_thsb(gt):
                """SBUF copy of theta (Act; tensor_tensor may read only one
                PSUM operand, so theta must transit SBUF before the gate)."""
                thsb = thsb_p.tile([128, B_CORE], F32R, tag="thsb")
                nc.scalar.copy(thsb[:], gt[:, B_CORE:])
                return thsb

            def mk_q(thsb, psE):
                """q = theta ⊙ psE as ONE broadcast DVE op."""
                q = q_p.tile([128, 2 * B_CORE], F32R, tag="q")
                nc.vector.tensor_mul(
                    q[:].rearrange("p (i c) -> p i c", i=2),
                    thsb[:].unsqueeze(1).broadcast_to((128, 2, B_CORE)),
                    psE[:].rearrange("p (i c) -> p i c", i=2))
                return q

            # ---- prologue: gate pipeline state for macros 0..3 from y0,
            # interleaved with warm-up junk on PE
            psE0 = psE_p.tile([128, 2 * B_CORE], F32, tag="psE")
            nc.tensor.matmul(psE0[:, 0:B_CORE], wts["PR0"], y[:],
                             start=True, stop=True)
            nc.tensor.matmul(psE0[:, B_CORE:], wts["PR1"], y[:],
                             start=True, stop=True)
            psE_next = psE_p.tile([128, 2 * B_CORE], F32, tag="psE")
            nc.tensor.matmul(psE_next[:, 0:B_CORE], wts["PA0u"], y[:],
                             start=True, stop=True)
            nc.tensor.matmul(psE_next[:, B_CORE:], wts["PA1u"], y[:],
                             start=True, stop=True)
            gA = psG_p.tile([128, 2 * B_CORE], F32, tag="g")
            sq0 = mk_sq("THP0", y, gA)
            gB = psG_p.tile([128, 2 * B_CORE], F32, tag="g")
            sq1 = mk_sq("THP1", y, gB)
            mk_fold(sq0, gA)
            mk_fold(sq1, gB)
            q_cur = mk_q(mk_thsb(gA), psE0)  # q(0)
            thsb_next = mk_thsb(gB)          # theta(1)
            # theta(2) tile: thsb copy happens inside iteration 0
            gC = psG_p.tile([128, 2 * B_CORE], F32, tag="g")
            g_prev = mk_fold(mk_sq("THP2", y, gC), gC)
            # seed for iteration 0's fold -> theta(3)
            gD = psG_p.tile([128, 2 * B_CORE], F32, tag="g")
            sq_prev = mk_sq("THP3", y, gD)

            y_prev = None
            for i in range(N_MACRO):
                # ---- gate ops for LATER macros first: every input below
                # was finished at least one iteration ago, so DVE starts
                # immediately while PE waits for y_i
                q_next = mk_q(thsb_next, psE_next)          # q(i+1)
                if i + 2 <= N_MACRO:
                    thsb_next = mk_thsb(g_prev)             # theta(i+2)
                # ---- state chain (critical path): consume q(i)
                chps_t = psCh_p.tile([128, B_CORE], F32, tag="ch")
                chps = chps_t[:]
                # q-gated matmuls FIRST (q is ready at iter start), the
                # y-gated propagator LAST: only M10@y sits on the y-cycle
                nc.tensor.matmul(chps, wts["C75"], q_cur[:, 0:B_CORE],
                                 start=True, stop=False)
                nc.tensor.matmul(chps, wts["C25"], q_cur[:, B_CORE:],
                                 start=False, stop=False)
                nc.tensor.matmul(chps, wts["M10"], y[:],
                                 start=False, stop=True)
                out_t = out_p.tile([128, 2 * B_CORE], F32R, tag="out")
                y_new = out_t[:, B_CORE:]
                nc.scalar.copy(y_new, chps)
                # ---- branch output (interval 2i+1)
                brps_t = psBr_p.tile([128, B_CORE], F32, tag="br")
                brps = brps_t[:]
                nc.tensor.matmul(brps, wts["C25"], q_cur[:, 0:B_CORE],
                                 start=True, stop=False)
                nc.tensor.matmul(brps, wts["M5"], y[:],
                                 start=False, stop=True)
                nc.vector.tensor_copy(out_t[:, 0:B_CORE], brps)
                # one DMA for both intervals; both APs flat [128,512] so the
                # DGE emits one 2KB descriptor per partition
                nc.sync.dma_start(
                    traj_d[:, 2 * i:2 * i + 2, :].rearrange("p i c -> p (i c)"),
                    out_t[:])
                # ---- gate pipeline for later macros
                psE_new = None
                if i + 2 <= N_MACRO:
                    psE_new = psE_p.tile([128, 2 * B_CORE], F32, tag="psE")
                    if i == 0:
                        nc.tensor.matmul(psE_new[:, 0:B_CORE], wts["PB0u"],
                                         y[:], start=True, stop=True)
                        nc.tensor.matmul(psE_new[:, B_CORE:], wts["PB1u"],
                                         y[:], start=True, stop=True)
                    else:
                        nc.tensor.matmul(psE_new[:, 0:B_CORE], wts["PA3"],
                                         y[:], start=True, stop=False)
                        nc.tensor.matmul(psE_new[:, 0:B_CORE], wts["PB2"],
                                         y_prev[:], start=False, stop=True)
                        nc.tensor.matmul(psE_new[:, B_CORE:], wts["PA3b"],
                                         y[:], start=True, stop=False)
                        nc.tensor.matmul(psE_new[:, B_CORE:], wts["PB2b"],
                                         y_prev[:], start=False, stop=True)
                # fold theta(i+3) from last iteration's sq; predict and
                # square for theta(i+4)
                gt = None
                if i + 3 <= N_MACRO:
                    gt = psG_p.tile([128, 2 * B_CORE], F32, tag="g")
                    mk_fold(sq_prev, gt)
                if i + 4 <= N_MACRO:
                    sq_prev = mk_sq("TH", y, gt)
                g_prev = gt
                q_cur = q_next
                psE_next = psE_new
                y_prev, y = y, y_new

            # ---- epilogue: final interval 99 (branch-style off y_49)
            brps_t = psBr_p.tile([128, B_CORE], F32, tag="br")
            brps = brps_t[:]
            nc.tensor.matmul(brps, wts["M5"], y[:],
                             start=True, stop=False)
            nc.tensor.matmul(brps, wts["C25"], q_cur[:, 0:B_CORE],
                             start=False, stop=True)
            out_t = out_p.tile([128, 2 * B_CORE], F32R, tag="out")
            nc.scalar.copy(out_t[:, 0:B_CORE], brps)
            nc.sync.dma_start(traj_d[:, n_intervals - 1, :],
                              out_t[:, 0:B_CORE])
    nc.compile()
    return nc


# ---------------------------------------------------------------- driver
_PROGRAM_CACHE = {}


def kernel(A0_real, A0_imag, params, biases_real, biases_imag,
           omega, kappa, nonlinearity):
    from concourse.bass_utils import run_bass_kernel_spmd

    NC_CORES = 8
    B = A0_real.shape[0]
    BS = B // NC_CORES
    assert BS == B_CORE, f"expected batch {NC_CORES * B_CORE}, got {B}"
    NI = N_INTERVALS_FULL

    wmats, idx = build_weights(np.asarray(params, np.float32),
                               np.asarray(kappa, np.float32),
                               np.asarray(omega, np.float32),
                               np.asarray(nonlinearity, np.float32))

    key = NI
    if key not in _PROGRAM_CACHE:
        _PROGRAM_CACHE[key] = build_kernel(NI, idx)
    nc = _PROGRAM_CACHE[key]

    in_maps = []
    for c in range(NC_CORES):
        sl = slice(c * BS, (c + 1) * BS)
        S0 = host_initial_state(np.asarray(A0_real[sl], np.float32),
                                np.asarray(A0_imag[sl], np.float32),
                                np.asarray(biases_real, np.float32),
                                np.asarray(biases_imag, np.float32))
        in_maps.append({"s0": S0, "wmats": wmats})

    res = run_bass_kernel_spmd(nc, in_maps, core_ids=list(range(NC_CORES)))

    out = np.empty((EVAL_PTS, B, MODES), np.complex64)
    for c in range(NC_CORES):
        sl = slice(c * BS, (c + 1) * BS)
        S0 = in_maps[c]["s0"]
        out[0, sl] = (S0[:MODES] + 1j * S0[MODES:]).T
        traj = res.results[c]["traj"]  # [128, NI, BS] fp32 (partition-major)
        out[1:, sl] = (traj[:MODES] + 1j * traj[MODES:]).transpose(1, 2, 0)
    return out
